# revision 9
# baseline (speedup 1.0000x reference)
"""Trainium2 Bass kernel for a 6-layer transformer encoder.

nn_Encoder: B=8, S=2048, D=512, NHEAD=8, D_FF=2048, fp32.

Strategy
--------
Pure data-parallel: one batch element per NeuronCore (8 cores), no
collectives. The reference's `reshape(B, NHEAD, S, D_HEAD)` WITHOUT
transpose makes attention block-diagonal over 8 slabs of 256 tokens:
each slab's (256 x 512) block is reinterpreted row-major as (2048 x 64)
and self-attends. Within a slab we permute the 2048 attention rows/cols
from (s_local*8 + j) to (j*256 + s_local) order, which makes every
matmul operand a natural slice of transposed activations; softmax is
permutation-invariant along both axes (applied consistently to V and
un-permuted on output).

All matmuls run in float32r (fp32 operands truncated to ~FP22 by the PE,
full bf16-rate at free-dim >= 256). Activations live transposed
(features on partitions) so layernorm statistics are computed with
ones-vector matmuls on the tensor engine; softmax denominators come for
free from a 65th "ones" column appended to V in the attn@V matmul.
exp() runs on the scalar engine over [128, 1024] PSUM mega-tiles.
"""

import numpy as np

P = 128
D = 512
S = 2048
FF = 2048
NH = 8          # slabs
T = 256         # tokens per slab
DH = 64
G = D // P      # 4
GF = FF // P    # 16
B = 8
EPS = 1e-5
N_LAYERS = 6

_CACHE = {}


def _build(n_layers=N_LAYERS, n_slabs=NH):
    import concourse.bass as bass
    import concourse.tile as tile
    from concourse import bacc, mybir

    F32R = mybir.dt.float32r
    F32 = mybir.dt.float32
    AF = mybir.ActivationFunctionType
    OP = mybir.AluOpType

    nc = bacc.Bacc("TRN2", target_bir_lowering=False)

    xT_d = nc.dram_tensor("xT", (D, S), F32R, kind="ExternalInput")
    wq_d = nc.dram_tensor("wq", (N_LAYERS, D, D), F32R, kind="ExternalInput")
    wk_d = nc.dram_tensor("wk", (N_LAYERS, D, D), F32R, kind="ExternalInput")
    wv_d = nc.dram_tensor("wv", (N_LAYERS, D, D), F32R, kind="ExternalInput")
    wo_d = nc.dram_tensor("wo", (N_LAYERS, D, D), F32R, kind="ExternalInput")
    w1_d = nc.dram_tensor("w1", (N_LAYERS, D, FF), F32R, kind="ExternalInput")
    w2_d = nc.dram_tensor("w2", (N_LAYERS, FF, D), F32R, kind="ExternalInput")
    b1_d = nc.dram_tensor("b1", (N_LAYERS, FF), F32R, kind="ExternalInput")
    b2_d = nc.dram_tensor("b2", (N_LAYERS, D), F32R, kind="ExternalInput")
    g1_d = nc.dram_tensor("g1", (N_LAYERS, D), F32, kind="ExternalInput")
    bt1_d = nc.dram_tensor("beta1", (N_LAYERS, D), F32, kind="ExternalInput")
    g2_d = nc.dram_tensor("g2", (N_LAYERS, D), F32, kind="ExternalInput")
    bt2_d = nc.dram_tensor("beta2", (N_LAYERS, D), F32, kind="ExternalInput")
    out_d = nc.dram_tensor("out", (D, S), F32R, kind="ExternalOutput")

    with tile.TileContext(nc) as tc:
        with tc.tile_pool(name="const", bufs=1) as cpool, \
             tc.tile_pool(name="x", bufs=1) as xpool, \
             tc.tile_pool(name="w", bufs=2) as wpool, \
             tc.tile_pool(name="par", bufs=1) as parpool, \
             tc.tile_pool(name="slab", bufs=2) as slab, \
             tc.tile_pool(name="slab1", bufs=1) as slab1, \
             tc.tile_pool(name="ff", bufs=1) as ffpool, \
             tc.tile_pool(name="pT", bufs=2) as ppool, \
             tc.tile_pool(name="sm", bufs=2) as small, \
             tc.tile_pool(name="um", bufs=2, space="PSUM") as umpool, \
             tc.tile_pool(name="po", bufs=1, space="PSUM") as opool, \
             tc.tile_pool(name="pp", bufs=2, space="PSUM") as pppool:

            # constants
            ones_f = cpool.tile([P, 16], F32, tag="ones_f")
            nc.vector.memset(ones_f, 1.0)
            ones128 = cpool.tile([P, 1], F32R, tag="ones128")   # stats lhsT
            nc.vector.tensor_copy(out=ones128, in_=ones_f[:, 0:1])
            ones_row = cpool.tile([1, T], F32R, tag="ones_row")  # bias-fold rhs
            nc.vector.tensor_copy(out=ones_row, in_=ones_f[0:1, 0:1].to_broadcast((1, T)))
            eps_t = cpool.tile([1, 1], F32, tag="eps")
            nc.vector.memset(eps_t, EPS)

            # persistent activations, transposed: xT_s[p, g, s] = x[s, g*128+p]
            xT_s = xpool.tile([P, G, S], F32R, tag="xT")
            for g in range(G):
                nc.sync.dma_start(xT_s[:, g, :], xT_d[g * P:(g + 1) * P, :])

            def layernorm(r, r2, gam, bet, dst):
                """LN over feature dim of r [P, G, T] (features on partitions,
                split across G chunks); writes dst = (r-mu)*rstd*gam + bet."""
                nc.vector.tensor_tensor(r2[:], r[:], r[:], OP.mult)
                st = pppool.tile([P, 512], F32, tag="pp")
                for g in range(G):
                    nc.tensor.matmul(st[0:1, 0:T], ones128, r[:, g, :],
                                     start=(g == 0), stop=(g == G - 1))
                for g in range(G):
                    nc.tensor.matmul(st[0:1, T:2 * T], ones128, r2[:, g, :],
                                     start=(g == 0), stop=(g == G - 1))
                ms = small.tile([1, 4, T], F32, tag="musd")  # mu, rstd, sd, ex2
                nc.vector.tensor_scalar_mul(ms[0:1, 0, :], st[0:1, 0:T], 1.0 / D)
                nc.vector.tensor_scalar_mul(ms[0:1, 3, :], st[0:1, T:2 * T], 1.0 / D)
                # var = ex2 - mu^2 -> slot 2
                nc.vector.tensor_tensor(ms[0:1, 2, :], ms[0:1, 0, :], ms[0:1, 0, :], OP.mult)
                nc.vector.tensor_tensor(ms[0:1, 2, :], ms[0:1, 3, :], ms[0:1, 2, :], OP.subtract)
                # var+eps in slot 3 (reuse), sd ~= sqrt(var+eps) via ACT (loose),
                # then one Newton step on rstd: y' = y*(1.5 - 0.5*(var+eps)*y^2)
                nc.vector.tensor_scalar(out=ms[0:1, 3, :], in0=ms[0:1, 2, :],
                                        scalar1=1.0, scalar2=EPS,
                                        op0=OP.mult, op1=OP.add)
                nc.scalar.activation(out=ms[0:1, 2, :], in_=ms[0:1, 2, :],
                                     func=AF.Sqrt, bias=eps_t[0:1, 0:1], scale=1.0)
                nc.vector.reciprocal(out=ms[0:1, 1, :], in_=ms[0:1, 2, :])
                nc.vector.tensor_tensor(ms[0:1, 2, :], ms[0:1, 1, :], ms[0:1, 1, :], OP.mult)
                nc.vector.tensor_tensor(ms[0:1, 2, :], ms[0:1, 3, :], ms[0:1, 2, :], OP.mult)
                nc.vector.tensor_scalar(out=ms[0:1, 2, :], in0=ms[0:1, 2, :],
                                        scalar1=-0.5, scalar2=1.5,
                                        op0=OP.mult, op1=OP.add)
                nc.vector.tensor_tensor(ms[0:1, 1, :], ms[0:1, 1, :], ms[0:1, 2, :], OP.mult)
                bc = small.tile([P, 2, T], F32, tag="mubc")
                nc.gpsimd.partition_broadcast(bc, ms[0:1, 0:2, :])
                mu_b = bc[:, 0:1, :].to_broadcast((P, G, T))
                sd_b = bc[:, 1:2, :].to_broadcast((P, G, T))
                nc.vector.tensor_tensor(r[:], r[:], mu_b, OP.subtract)
                nc.vector.tensor_tensor(r[:], r[:], sd_b, OP.mult)
                gam_b = gam[:, :, None].to_broadcast((P, G, T))
                bet_b = bet[:, :, None].to_broadcast((P, G, T))
                nc.vector.tensor_tensor(r[:], r[:], gam_b, OP.mult)
                nc.vector.tensor_tensor(dst, r[:], bet_b, OP.add)

            def av(poE, poO, v65, pE, pO, q):
                """attn@V matmuls for one exp'd quarter (4 b-chunks)."""
                for i in range(4):
                    bidx = 4 * q + i
                    jb, c = bidx // 2, bidx % 2
                    nc.tensor.matmul(poE[0:65, :], v65[:, c, jb, :],
                                     pE[:, i * T:(i + 1) * T],
                                     start=(bidx == 0), stop=(bidx == 15))
                    nc.tensor.matmul(poO[0:65, :], v65[:, c, jb, :],
                                     pO[:, i * T:(i + 1) * T],
                                     start=(bidx == 0), stop=(bidx == 15))

            for l in range(n_layers):
                # ---- attention weights: q|k|v|o concat, [P, ki, 4*512] ----
                wa = wpool.tile([P, G, 4 * D], F32R, tag="w")
                for wi, w_d in enumerate((wq_d, wk_d, wv_d, wo_d)):
                    for ki in range(G):
                        nc.sync.dma_start(wa[:, ki, wi * D:(wi + 1) * D],
                                          w_d[l, ki * P:(ki + 1) * P, :])
                QO, KO, VO, OO = 0, D, 2 * D, 3 * D
                g1s = parpool.tile([P, G], F32, tag="g1")
                bt1s = parpool.tile([P, G], F32, tag="bt1")
                g2s = parpool.tile([P, G], F32, tag="g2")
                bt2s = parpool.tile([P, G], F32, tag="bt2")
                for t_, d_ in ((g1s, g1_d), (bt1s, bt1_d), (g2s, g2_d), (bt2s, bt2_d)):
                    nc.sync.dma_start(t_, d_[l].rearrange("(o p) -> p o", p=P))
                b1r = parpool.tile([1, FF], F32R, tag="b1r")
                nc.sync.dma_start(b1r, b1_d[l:l + 1, :])
                b2r = parpool.tile([1, D], F32R, tag="b2r")
                nc.sync.dma_start(b2r, b2_d[l:l + 1, :])

                # ======== ATTENTION PHASE (all slabs) ========
                for h in range(n_slabs):
                    ts = slice(h * T, (h + 1) * T)
                    qT = slab.tile([P, G, T], F32R, tag="qT")
                    ks = slab1.tile([P, 8, T], F32R, tag="ks")
                    v65 = slab.tile([P, 2, 8, 65], F32R, tag="v65")
                    nc.vector.tensor_copy(out=v65[:, :, :, 64:65], in_=ones_f)
                    for gg in range(2):
                        pq = pppool.tile([P, 512], F32, tag="pp")
                        for g2 in range(2):
                            g = gg * 2 + g2
                            for ki in range(G):
                                nc.tensor.matmul(
                                    pq[:, g2 * T:(g2 + 1) * T],
                                    wa[:, ki, QO + g * P:QO + (g + 1) * P],
                                    xT_s[:, ki, ts],
                                    start=(ki == 0), stop=(ki == G - 1))
                        nc.vector.tensor_copy(out=qT[:, 2 * gg:2 * gg + 2, :], in_=pq)
                    for gg in range(2):
                        pk = pppool.tile([P, 512], F32, tag="pp")
                        for g2 in range(2):
                            g = gg * 2 + g2
                            for ki in range(G):
                                nc.tensor.matmul(
                                    pk[:, g2 * T:(g2 + 1) * T],
                                    wa[:, ki, KO + g * P:KO + (g + 1) * P],
                                    xT_s[:, ki, ts],
                                    start=(ki == 0), stop=(ki == G - 1))
                        for g2 in range(2):
                            g = gg * 2 + g2
                            lo = pk[0:64, g2 * T:(g2 + 1) * T]
                            hi = pk[64:128, g2 * T:(g2 + 1) * T]
                            nc.vector.tensor_copy(out=ks[0:64, 2 * g, :], in_=lo)
                            nc.vector.tensor_copy(out=ks[64:128, 2 * g, :], in_=lo)
                            nc.vector.tensor_copy(out=ks[0:64, 2 * g + 1, :], in_=hi)
                            nc.vector.tensor_copy(out=ks[64:128, 2 * g + 1, :], in_=hi)
                    for c in range(2):
                        pv = pppool.tile([P, 512], F32, tag="pp")
                        for ki in range(G):
                            nc.tensor.matmul(
                                pv, xT_s[:, ki, h * T + c * P:h * T + (c + 1) * P],
                                wa[:, ki, VO:VO + D],
                                start=(ki == 0), stop=(ki == G - 1))
                        nc.vector.tensor_copy(out=v65[:, c, :, 0:64], in_=pv)

                    # ---- attention: a-chunk pairs (j_a = 2m, 2m+1) ----
                    oT = slab.tile([P, G, T], F32R, tag="oT")
                    for m in range(4):
                        rhsE = qT[0:64, m, :]
                        rhsO = qT[64:128, m, :]
                        poE = opool.tile([P, 256], F32, tag="poE")
                        poO = opool.tile([P, 256], F32, tag="poO")
                        prev = None
                        for q in range(4):
                            umE = umpool.tile([P, 1024], F32, tag="um")
                            umO = umpool.tile([P, 1024], F32, tag="um")
                            for i in range(4):
                                bidx = 4 * q + i
                                jb, c = bidx // 2, bidx % 2
                                lhsE = ks[0:64, jb, c * P:(c + 1) * P]
                                lhsO = ks[64:128, jb, c * P:(c + 1) * P]
                                nc.tensor.matmul(umE[:, i * T:(i + 1) * T], lhsE, rhsE,
                                                 start=True, stop=True)
                                nc.tensor.matmul(umO[:, i * T:(i + 1) * T], lhsO, rhsO,
                                                 start=True, stop=True)
                            pE = ppool.tile([P, 1024], F32R, tag="pT")
                            pO = ppool.tile([P, 1024], F32R, tag="pT")
                            nc.scalar.activation(out=pE, in_=umE, func=AF.Exp, scale=0.125)
                            nc.scalar.activation(out=pO, in_=umO, func=AF.Exp, scale=0.125)
                            if prev is not None:
                                av(poE, poO, v65, *prev)
                            prev = (pE, pO, q)
                        av(poE, poO, v65, *prev)
                        rec = small.tile([1, 512], F32, tag="rec")
                        nc.vector.reciprocal(out=rec[:, 0:T], in_=poE[64:65, :])
                        nc.vector.reciprocal(out=rec[:, T:2 * T], in_=poO[64:65, :])
                        bc = small.tile([64, 512], F32, tag="bc")
                        nc.gpsimd.partition_broadcast(bc, rec)
                        nc.vector.tensor_tensor(oT[0:64, m, :], poE[0:64, :],
                                                bc[0:64, 0:T], OP.mult)
                        nc.vector.tensor_tensor(oT[64:128, m, :], poO[0:64, :],
                                                bc[0:64, T:2 * T], OP.mult)

                    # ---- wo projection + residual + LN1 ----
                    r = slab.tile([P, G, T], F32R, tag="r")
                    r2 = slab1.tile([P, G, T], F32R, tag="r2")
                    for gg in range(2):
                        py = pppool.tile([P, 512], F32, tag="pp")
                        for g2 in range(2):
                            g = gg * 2 + g2
                            for gi in range(G):
                                nc.tensor.matmul(
                                    py[:, g2 * T:(g2 + 1) * T],
                                    wa[:, gi, OO + g * P:OO + (g + 1) * P],
                                    oT[:, gi, :],
                                    start=(gi == 0), stop=(gi == G - 1))
                        nc.vector.tensor_tensor(r[:, 2 * gg:2 * gg + 2, :], py,
                                                xT_s[:, 2 * gg:2 * gg + 2, ts], OP.add)
                    layernorm(r, r2, g1s, bt1s, xT_s[:, :, ts])

                # ======== FFN PHASE (all slabs) ========
                w1s = wpool.tile([P, G, FF], F32R, tag="w")
                for ki in range(G):
                    nc.sync.dma_start(w1s[:, ki, :], w1_d[l, ki * P:(ki + 1) * P, :])
                w2s = wpool.tile([P, G, FF], F32R, tag="w")  # fi chunk at [:, fi//4, (fi%4)*512]
                for fi in range(GF):
                    nc.sync.dma_start(w2s[:, fi // 4, (fi % 4) * D:(fi % 4 + 1) * D],
                                      w2_d[l, fi * P:(fi + 1) * P, :])

                for h in range(n_slabs):
                    ts = slice(h * T, (h + 1) * T)
                    r = slab.tile([P, G, T], F32R, tag="r")
                    r2 = slab1.tile([P, G, T], F32R, tag="r2")
                    fT = ffpool.tile([P, GF, T], F32R, tag="fT")
                    for q in range(4):
                        um = umpool.tile([P, 1024], F32, tag="um")
                        for i in range(4):
                            dk = 4 * q + i
                            nc.tensor.matmul(um[:, i * T:(i + 1) * T],
                                             b1r[0:1, dk * P:(dk + 1) * P], ones_row,
                                             start=True, stop=False)
                            for ki in range(G):
                                nc.tensor.matmul(
                                    um[:, i * T:(i + 1) * T],
                                    w1s[:, ki, dk * P:(dk + 1) * P],
                                    xT_s[:, ki, ts],
                                    start=False, stop=(ki == G - 1))
                        nc.scalar.activation(out=fT[:, 4 * q:4 * q + 4, :], in_=um,
                                             func=AF.Relu, scale=1.0)
                    for gg in range(2):
                        py = pppool.tile([P, 512], F32, tag="pp")
                        for g2 in range(2):
                            g = gg * 2 + g2
                            nc.tensor.matmul(py[:, g2 * T:(g2 + 1) * T],
                                             b2r[0:1, g * P:(g + 1) * P], ones_row,
                                             start=True, stop=False)
                            for fi in range(GF):
                                nc.tensor.matmul(
                                    py[:, g2 * T:(g2 + 1) * T],
                                    w2s[:, fi // 4, (fi % 4) * D + g * P:(fi % 4) * D + (g + 1) * P],
                                    fT[:, fi, :],
                                    start=False, stop=(fi == GF - 1))
                        nc.vector.tensor_tensor(r[:, 2 * gg:2 * gg + 2, :], py,
                                                xT_s[:, 2 * gg:2 * gg + 2, ts], OP.add)
                    layernorm(r, r2, g2s, bt2s, xT_s[:, :, ts])

            for g in range(G):
                nc.sync.dma_start(out_d[g * P:(g + 1) * P, :], xT_s[:, g, :])

    nc.compile()
    return nc


def _get_nc(n_layers=N_LAYERS, n_slabs=NH):
    key = (n_layers, n_slabs)
    if key not in _CACHE:
        _CACHE[key] = _build(n_layers, n_slabs)
    return _CACHE[key]


def kernel(x, wq, wk, wv, wo, w1, b1, w2, b2, g1, beta1, g2, beta2,
           _n_layers=N_LAYERS, _n_slabs=NH, _trace=False):
    from concourse.bass_utils import run_bass_kernel_spmd

    nc = _get_nc(_n_layers, _n_slabs)
    x = np.asarray(x, dtype=np.float32)
    common = {
        "wq": np.ascontiguousarray(np.asarray(wq, np.float32)),
        "wk": np.ascontiguousarray(np.asarray(wk, np.float32)),
        "wv": np.ascontiguousarray(np.asarray(wv, np.float32)),
        "wo": np.ascontiguousarray(np.asarray(wo, np.float32)),
        "w1": np.ascontiguousarray(np.asarray(w1, np.float32)),
        "w2": np.ascontiguousarray(np.asarray(w2, np.float32)),
        "b1": np.ascontiguousarray(np.asarray(b1, np.float32)),
        "b2": np.ascontiguousarray(np.asarray(b2, np.float32)),
        "g1": np.ascontiguousarray(np.asarray(g1, np.float32)),
        "beta1": np.ascontiguousarray(np.asarray(beta1, np.float32)),
        "g2": np.ascontiguousarray(np.asarray(g2, np.float32)),
        "beta2": np.ascontiguousarray(np.asarray(beta2, np.float32)),
    }
    in_maps = [{"xT": np.ascontiguousarray(x[b].T), **common} for b in range(B)]
    res = run_bass_kernel_spmd(nc, in_maps, core_ids=list(range(B)), trace=_trace)
    out = np.stack([res.results[b]["out"].T for b in range(B)])
    if _trace:
        kernel.last_exec_time_ns = res.exec_time_ns
        kernel.last_mean_exec_time_ns = res.mean_exec_time_ns
        kernel.last_results = res
    return out.astype(np.float32)


# revision 24
# speedup vs baseline: 1.1876x; 1.1876x over previous
"""Trainium2 Bass kernel for a 6-layer transformer encoder.

nn_Encoder: B=8, S=2048, D=512, NHEAD=8, D_FF=2048, fp32.

Strategy
--------
Pure data-parallel: one batch element per NeuronCore (8 cores), no
collectives. The reference's `reshape(B, NHEAD, S, D_HEAD)` WITHOUT
transpose makes attention block-diagonal over 8 slabs of 256 tokens:
each slab's (256 x 512) block is reinterpreted row-major as (2048 x 64)
and self-attends. Within a slab we permute the 2048 attention rows/cols
from (s_local*8 + j) to (j*256 + s_local) order, which makes every
matmul operand a natural slice of transposed activations; softmax is
permutation-invariant along both axes (applied consistently to V and
un-permuted on output).

All matmuls run in float32r (fp32 operands truncated to ~FP22 by the PE,
full bf16-rate at free-dim >= 256). Activations live transposed
(features on partitions) so layernorm statistics are computed with
ones-vector matmuls on the tensor engine; softmax denominators come for
free from a 65th "ones" column appended to V in the attn@V matmul.
exp() runs on the scalar engine over [128, 1024] PSUM mega-tiles.
"""

import numpy as np

P = 128
D = 512
S = 2048
FF = 2048
NH = 8          # slabs
T = 256         # tokens per slab
DH = 64
G = D // P      # 4
GF = FF // P    # 16
B = 8
EPS = 1e-5
N_LAYERS = 6

_CACHE = {}


def _build(n_layers=N_LAYERS, n_slabs=NH):
    import concourse.bass as bass
    import concourse.tile as tile
    from concourse import bacc, mybir

    F32R = mybir.dt.float32r
    F32 = mybir.dt.float32
    U32 = mybir.dt.uint32
    AF = mybir.ActivationFunctionType
    OP = mybir.AluOpType

    nc = bacc.Bacc("TRN2", target_bir_lowering=False)

    xT_d = nc.dram_tensor("xT", (D, S), F32R, kind="ExternalInput")
    wq_d = nc.dram_tensor("wq", (N_LAYERS, D, D), F32R, kind="ExternalInput")
    wk_d = nc.dram_tensor("wk", (N_LAYERS, D, D), F32R, kind="ExternalInput")
    wv_d = nc.dram_tensor("wv", (N_LAYERS, D, D), F32R, kind="ExternalInput")
    wo_d = nc.dram_tensor("wo", (N_LAYERS, D, D), F32R, kind="ExternalInput")
    w1_d = nc.dram_tensor("w1", (N_LAYERS, D, FF), F32R, kind="ExternalInput")
    w2_d = nc.dram_tensor("w2", (N_LAYERS, FF, D), F32R, kind="ExternalInput")
    b1_d = nc.dram_tensor("b1", (N_LAYERS, FF), F32R, kind="ExternalInput")
    b2_d = nc.dram_tensor("b2", (N_LAYERS, D), F32R, kind="ExternalInput")
    g1_d = nc.dram_tensor("g1", (N_LAYERS, D), F32, kind="ExternalInput")
    bt1_d = nc.dram_tensor("beta1", (N_LAYERS, D), F32, kind="ExternalInput")
    g2_d = nc.dram_tensor("g2", (N_LAYERS, D), F32, kind="ExternalInput")
    bt2_d = nc.dram_tensor("beta2", (N_LAYERS, D), F32, kind="ExternalInput")
    out_d = nc.dram_tensor("out", (D, S), F32R, kind="ExternalOutput")

    with tile.TileContext(nc) as tc:
        with tc.tile_pool(name="const", bufs=1) as cpool, \
             tc.tile_pool(name="x", bufs=1) as xpool, \
             tc.tile_pool(name="w", bufs=2) as wpool, \
             tc.tile_pool(name="par", bufs=1) as parpool, \
             tc.tile_pool(name="slab", bufs=2) as slab, \
             tc.tile_pool(name="slab1", bufs=1) as slab1, \
             tc.tile_pool(name="ff", bufs=1) as ffpool, \
             tc.tile_pool(name="pT", bufs=2) as ppool, \
             tc.tile_pool(name="sm", bufs=2) as small, \
             tc.tile_pool(name="um", bufs=2, space="PSUM") as umpool, \
             tc.tile_pool(name="po", bufs=1, space="PSUM") as opool, \
             tc.tile_pool(name="pp", bufs=2, space="PSUM") as pppool:

            # constants
            ones_f = cpool.tile([P, 16], F32, tag="ones_f")
            nc.vector.memset(ones_f, 1.0)
            ones128 = cpool.tile([P, 1], F32R, tag="ones128")   # stats lhsT
            nc.vector.tensor_copy(out=ones128, in_=ones_f[:, 0:1])
            ones_row = cpool.tile([1, T], F32R, tag="ones_row")  # bias-fold rhs
            nc.vector.tensor_copy(out=ones_row, in_=ones_f[0:1, 0:1].to_broadcast((1, T)))
            eps_t = cpool.tile([1, 1], F32, tag="eps")
            nc.vector.memset(eps_t, EPS)
            ones_row2 = cpool.tile([1, 2 * T], F32R, tag="ones_row2")
            nc.vector.tensor_copy(out=ones_row2, in_=ones_f[0:1, 0:1].to_broadcast((1, 2 * T)))
            zf = cpool.tile([1, 65], F32, tag="zf")
            nc.vector.memset(zf, 0.0)
            zeros65 = cpool.tile([1, 65], F32R, tag="zeros65")
            nc.vector.tensor_copy(out=zeros65, in_=zf)

            # persistent activations, transposed: xT_s[p, g, s] = x[s, g*128+p]
            xT_s = xpool.tile([P, G, S], F32R, tag="xT")
            for g in range(G):
                nc.sync.dma_start(xT_s[:, g, :], xT_d[g * P:(g + 1) * P, :])

            def layernorm(r, r2, gam, bet, dst):
                """LN over feature dim of r [P, G, T] (features on partitions,
                split across G chunks); writes dst = (r-mu)*rstd*gam + bet."""
                nc.vector.tensor_tensor(r2[:], r[:], r[:], OP.mult)
                st = pppool.tile([P, 512], F32, tag="pp")
                for g in range(G):
                    nc.tensor.matmul(st[0:1, 0:T], ones128, r[:, g, :],
                                     start=(g == 0), stop=(g == G - 1))
                for g in range(G):
                    nc.tensor.matmul(st[0:1, T:2 * T], ones128, r2[:, g, :],
                                     start=(g == 0), stop=(g == G - 1))
                ms = small.tile([1, 4, T], F32, tag="musd")  # mu, rstd, sd, ex2
                nc.vector.tensor_scalar_mul(ms[0:1, 0, :], st[0:1, 0:T], 1.0 / D)
                nc.vector.tensor_scalar_mul(ms[0:1, 3, :], st[0:1, T:2 * T], 1.0 / D)
                # var = ex2 - mu^2 -> slot 2
                nc.vector.tensor_tensor(ms[0:1, 2, :], ms[0:1, 0, :], ms[0:1, 0, :], OP.mult)
                nc.vector.tensor_tensor(ms[0:1, 2, :], ms[0:1, 3, :], ms[0:1, 2, :], OP.subtract)
                # rstd = rsqrt(var+eps) entirely on DVE: Quake-III bit-trick
                # seed + 2 Newton iterations (keeps ACT exp-only -> a single
                # activation-table load for the whole kernel).
                nc.vector.tensor_scalar(out=ms[0:1, 3, :], in0=ms[0:1, 2, :],
                                        scalar1=1.0, scalar2=EPS,
                                        op0=OP.mult, op1=OP.add)
                s1 = ms[0:1, 1, :]
                s2 = ms[0:1, 2, :]
                s3 = ms[0:1, 3, :]
                # seed bits = 0x5F3759DF - (v_bits >> 1), done in float space
                nc.vector.tensor_copy(out=s2, in_=s3.bitcast(U32))
                nc.vector.tensor_scalar(out=s2, in0=s2,
                                        scalar1=-0.5, scalar2=float(0x5F3759DF),
                                        op0=OP.mult, op1=OP.add)
                nc.vector.tensor_copy(out=s2.bitcast(U32), in_=s2)
                nc.vector.tensor_tensor(s1, s2, s2, OP.mult)
                nc.vector.tensor_tensor(s1, s3, s1, OP.mult)
                nc.vector.tensor_scalar(out=s1, in0=s1, scalar1=-0.5, scalar2=1.5,
                                        op0=OP.mult, op1=OP.add)
                nc.vector.tensor_tensor(s2, s2, s1, OP.mult)
                nc.vector.tensor_tensor(s1, s2, s2, OP.mult)
                nc.vector.tensor_tensor(s1, s3, s1, OP.mult)
                nc.vector.tensor_scalar(out=s1, in0=s1, scalar1=-0.5, scalar2=1.5,
                                        op0=OP.mult, op1=OP.add)
                nc.vector.tensor_tensor(s1, s2, s1, OP.mult)
                bc = small.tile([P, 2, T], F32, tag="mubc")
                nc.gpsimd.partition_broadcast(bc, ms[0:1, 0:2, :])
                mu_b = bc[:, 0:1, :].to_broadcast((P, G, T))
                sd_b = bc[:, 1:2, :].to_broadcast((P, G, T))
                nc.vector.tensor_tensor(r[:], r[:], mu_b, OP.subtract)
                nc.vector.tensor_tensor(r[:], r[:], sd_b, OP.mult)
                gam_b = gam[:, :, None].to_broadcast((P, G, T))
                bet_b = bet[:, :, None].to_broadcast((P, G, T))
                nc.gpsimd.tensor_tensor(r[:], r[:], gam_b, OP.mult)
                nc.gpsimd.tensor_tensor(dst, r[:], bet_b, OP.add)

            def av(poE, poO, v65, pE, pO, q):
                """attn@V matmuls for one exp'd quarter (4 b-chunks)."""
                for i in range(4):
                    bidx = 4 * q + i
                    jb, c = bidx // 2, bidx % 2
                    nc.tensor.matmul(poE[0:65, :], v65[:, c, jb, :],
                                     pE[:, i * T:(i + 1) * T],
                                     start=(bidx == 0), stop=(bidx == 15))
                    nc.tensor.matmul(poO[0:65, :], v65[:, c, jb, :],
                                     pO[:, i * T:(i + 1) * T],
                                     start=(bidx == 0), stop=(bidx == 15))

            for l in range(n_layers):
                # ---- attention weights: q|k|v|o concat, [P, ki, 4*512] ----
                wa = wpool.tile([P, G, 4 * D], F32R, tag="w")
                for wi, w_d in enumerate((wq_d, wk_d, wv_d, wo_d)):
                    for ki in range(G):
                        nc.sync.dma_start(wa[:, ki, wi * D:(wi + 1) * D],
                                          w_d[l, ki * P:(ki + 1) * P, :])
                QO, KO, VO, OO = 0, D, 2 * D, 3 * D
                g1s = parpool.tile([P, G], F32, tag="g1")
                bt1s = parpool.tile([P, G], F32, tag="bt1")
                g2s = parpool.tile([P, G], F32, tag="g2")
                bt2s = parpool.tile([P, G], F32, tag="bt2")
                for t_, d_ in ((g1s, g1_d), (bt1s, bt1_d), (g2s, g2_d), (bt2s, bt2_d)):
                    nc.sync.dma_start(t_, d_[l].rearrange("(o p) -> p o", p=P))
                b1r = parpool.tile([1, FF], F32R, tag="b1r")
                nc.sync.dma_start(b1r, b1_d[l:l + 1, :])
                b2r = parpool.tile([1, D], F32R, tag="b2r")
                nc.sync.dma_start(b2r, b2_d[l:l + 1, :])

                # ======== ATTENTION PHASE (all slabs) ========
                # Projections for slab h+1 are emitted piecewise between the
                # attention m-units of slab h, so the PE has work while the
                # scalar engine grinds through slab h's exp() stream.
                def alloc_proj(h):
                    qT = slab.tile([P, G, T], F32R, tag="qT", name=f"qT_{l}_{h}")
                    kT = slab.tile([P, G, T], F32R, tag="kT", name=f"kT_{l}_{h}")
                    ksx = slab.tile([P, G, T], F32R, tag="ksx", name=f"ksx_{l}_{h}")
                    v65 = slab.tile([P, 2, 8, 65], F32R, tag="v65", name=f"v65_{l}_{h}")
                    nc.vector.tensor_copy(out=v65[:, :, :, 64:65], in_=ones_f)
                    return qT, kT, ksx, v65

                def proj_pieces(h, tiles_h):
                    qT, kT, ksx, v65 = tiles_h
                    ts = slice(h * T, (h + 1) * T)
                    def qp(gg):
                        pq = pppool.tile([P, 512], F32, tag="pp")
                        for g2 in range(2):
                            g = gg * 2 + g2
                            for ki in range(G):
                                nc.tensor.matmul(
                                    pq[:, g2 * T:(g2 + 1) * T],
                                    wa[:, ki, QO + g * P:QO + (g + 1) * P],
                                    xT_s[:, ki, ts],
                                    start=(ki == 0), stop=(ki == G - 1))
                        nc.vector.tensor_copy(out=qT[:, 2 * gg:2 * gg + 2, :], in_=pq)
                    def kp(gg):
                        pk = pppool.tile([P, 512], F32, tag="pp")
                        for g2 in range(2):
                            g = gg * 2 + g2
                            for ki in range(G):
                                nc.tensor.matmul(
                                    pk[:, g2 * T:(g2 + 1) * T],
                                    wa[:, ki, KO + g * P:KO + (g + 1) * P],
                                    xT_s[:, ki, ts],
                                    start=(ki == 0), stop=(ki == G - 1))
                        nc.vector.tensor_copy(out=kT[:, 2 * gg:2 * gg + 2, :], in_=pk)
                        for g2 in range(2):
                            g = gg * 2 + g2
                            nc.sync.dma_start(ksx[0:64, g, :], kT[64:128, g, :])
                            nc.sync.dma_start(ksx[64:128, g, :], kT[0:64, g, :])
                    def vp(c):
                        pv = pppool.tile([P, 512], F32, tag="pp")
                        for ki in range(G):
                            nc.tensor.matmul(
                                pv, xT_s[:, ki, h * T + c * P:h * T + (c + 1) * P],
                                wa[:, ki, VO:VO + D],
                                start=(ki == 0), stop=(ki == G - 1))
                        nc.vector.tensor_copy(out=v65[:, c, :, 0:64], in_=pv)
                    return [lambda gg=gg: qp(gg) for gg in range(2)] + \
                           [lambda gg=gg: kp(gg) for gg in range(2)] + \
                           [lambda c=c: vp(c) for c in range(2)]

                tiles_h = alloc_proj(0)
                for piece in proj_pieces(0, tiles_h):
                    piece()
                for h in range(n_slabs):
                    ts = slice(h * T, (h + 1) * T)
                    if h + 1 < n_slabs:
                        tiles_n = alloc_proj(h + 1)
                        pending = proj_pieces(h + 1, tiles_n)
                    else:
                        tiles_n, pending = None, []
                    qT, kT, ksx, v65 = tiles_h

                    oT = slab.tile([P, G, T], F32R, tag="oT")
                    for m in range(4):
                        rhsE = qT[0:64, m, :]
                        rhsO = qT[64:128, m, :]
                        poE = opool.tile([P, 256], F32, tag="poE")
                        poO = opool.tile([P, 256], F32, tag="poO")
                        prev = None
                        for q in range(4):
                            umE = umpool.tile([P, 1024], F32, tag="um")
                            umO = umpool.tile([P, 1024], F32, tag="um")
                            for i in range(4):
                                bidx = 4 * q + i
                                jb, c = bidx // 2, bidx % 2
                                if jb % 2 == 0:
                                    lhsE = kT[0:64, jb // 2, c * P:(c + 1) * P]
                                    lhsO = ksx[64:128, jb // 2, c * P:(c + 1) * P]
                                else:
                                    lhsE = ksx[0:64, jb // 2, c * P:(c + 1) * P]
                                    lhsO = kT[64:128, jb // 2, c * P:(c + 1) * P]
                                nc.tensor.matmul(umE[:, i * T:(i + 1) * T], lhsE, rhsE,
                                                 start=True, stop=True)
                                nc.tensor.matmul(umO[:, i * T:(i + 1) * T], lhsO, rhsO,
                                                 start=True, stop=True)
                            pE = ppool.tile([P, 1024], F32R, tag="pT")
                            pO = ppool.tile([P, 1024], F32R, tag="pT")
                            nc.scalar.activation(out=pE, in_=umE, func=AF.Exp, scale=0.125)
                            nc.scalar.activation(out=pO, in_=umO, func=AF.Exp, scale=0.125)
                            if prev is not None:
                                av(poE, poO, v65, *prev)
                            prev = (pE, pO, q)
                        av(poE, poO, v65, *prev)
                        # fill the exp-bound stretch with next slab's projections
                        if pending:
                            pending.pop(0)()
                        if m >= 2 and pending:
                            pending.pop(0)()
                        rec = small.tile([1, 512], F32, tag="rec")
                        nc.vector.reciprocal(out=rec[:, 0:T], in_=poE[64:65, :])
                        nc.vector.reciprocal(out=rec[:, T:2 * T], in_=poO[64:65, :])
                        bc = small.tile([64, 512], F32, tag="bc")
                        nc.gpsimd.partition_broadcast(bc, rec)
                        nc.vector.tensor_tensor(oT[0:64, m, :], poE[0:64, :],
                                                bc[0:64, 0:T], OP.mult)
                        nc.vector.tensor_tensor(oT[64:128, m, :], poO[0:64, :],
                                                bc[0:64, T:2 * T], OP.mult)
                    for piece in pending:
                        piece()
                    tiles_h = tiles_n

                    # ---- wo projection + residual + LN1 ----
                    r = slab.tile([P, G, T], F32R, tag="r")
                    r2 = slab1.tile([P, G, T], F32R, tag="r2")
                    for gg in range(2):
                        py = pppool.tile([P, 512], F32, tag="pp")
                        for g2 in range(2):
                            g = gg * 2 + g2
                            for gi in range(G):
                                nc.tensor.matmul(
                                    py[:, g2 * T:(g2 + 1) * T],
                                    wa[:, gi, OO + g * P:OO + (g + 1) * P],
                                    oT[:, gi, :],
                                    start=(gi == 0), stop=(gi == G - 1))
                        nc.vector.tensor_tensor(r[:, 2 * gg:2 * gg + 2, :], py,
                                                xT_s[:, 2 * gg:2 * gg + 2, ts], OP.add)
                    layernorm(r, r2, g1s, bt1s, xT_s[:, :, ts])

                # ======== FFN PHASE (all slabs) ========
                w1s = wpool.tile([P, G, FF], F32R, tag="w")
                for ki in range(G):
                    nc.sync.dma_start(w1s[:, ki, :], w1_d[l, ki * P:(ki + 1) * P, :])
                w2s = wpool.tile([P, G, FF], F32R, tag="w")  # fi chunk at [:, fi//4, (fi%4)*512]
                for fi in range(GF):
                    nc.sync.dma_start(w2s[:, fi // 4, (fi % 4) * D:(fi % 4 + 1) * D],
                                      w2_d[l, fi * P:(fi + 1) * P, :])

                for h in range(n_slabs):
                    ts = slice(h * T, (h + 1) * T)
                    r = slab.tile([P, G, T], F32R, tag="r")
                    r2 = slab1.tile([P, G, T], F32R, tag="r2")
                    fT = ffpool.tile([P, GF, T], F32R, tag="fT")
                    for q in range(4):
                        um = umpool.tile([P, 1024], F32, tag="um")
                        for i in range(4):
                            dk = 4 * q + i
                            nc.tensor.matmul(um[:, i * T:(i + 1) * T],
                                             b1r[0:1, dk * P:(dk + 1) * P], ones_row,
                                             start=True, stop=False)
                            for ki in range(G):
                                nc.tensor.matmul(
                                    um[:, i * T:(i + 1) * T],
                                    w1s[:, ki, dk * P:(dk + 1) * P],
                                    xT_s[:, ki, ts],
                                    start=False, stop=(ki == G - 1))
                        nc.scalar.activation(out=fT[:, 4 * q:4 * q + 4, :], in_=um,
                                             func=AF.Relu, scale=1.0)
                    for gg in range(2):
                        py = pppool.tile([P, 512], F32, tag="pp")
                        for g2 in range(2):
                            g = gg * 2 + g2
                            nc.tensor.matmul(py[:, g2 * T:(g2 + 1) * T],
                                             b2r[0:1, g * P:(g + 1) * P], ones_row,
                                             start=True, stop=False)
                            for fi in range(GF):
                                nc.tensor.matmul(
                                    py[:, g2 * T:(g2 + 1) * T],
                                    w2s[:, fi // 4, (fi % 4) * D + g * P:(fi % 4) * D + (g + 1) * P],
                                    fT[:, fi, :],
                                    start=False, stop=(fi == GF - 1))
                        nc.vector.tensor_tensor(r[:, 2 * gg:2 * gg + 2, :], py,
                                                xT_s[:, 2 * gg:2 * gg + 2, ts], OP.add)
                    layernorm(r, r2, g2s, bt2s, xT_s[:, :, ts])

            for g in range(G):
                nc.sync.dma_start(out_d[g * P:(g + 1) * P, :], xT_s[:, g, :])

    nc.compile()
    return nc


def _get_nc(n_layers=N_LAYERS, n_slabs=NH):
    key = (n_layers, n_slabs)
    if key not in _CACHE:
        _CACHE[key] = _build(n_layers, n_slabs)
    return _CACHE[key]


def kernel(x, wq, wk, wv, wo, w1, b1, w2, b2, g1, beta1, g2, beta2,
           _n_layers=N_LAYERS, _n_slabs=NH, _trace=False):
    from concourse.bass_utils import run_bass_kernel_spmd

    nc = _get_nc(_n_layers, _n_slabs)
    x = np.asarray(x, dtype=np.float32)
    common = {
        "wq": np.ascontiguousarray(np.asarray(wq, np.float32)),
        "wk": np.ascontiguousarray(np.asarray(wk, np.float32)),
        "wv": np.ascontiguousarray(np.asarray(wv, np.float32)),
        "wo": np.ascontiguousarray(np.asarray(wo, np.float32)),
        "w1": np.ascontiguousarray(np.asarray(w1, np.float32)),
        "w2": np.ascontiguousarray(np.asarray(w2, np.float32)),
        "b1": np.ascontiguousarray(np.asarray(b1, np.float32)),
        "b2": np.ascontiguousarray(np.asarray(b2, np.float32)),
        "g1": np.ascontiguousarray(np.asarray(g1, np.float32)),
        "beta1": np.ascontiguousarray(np.asarray(beta1, np.float32)),
        "g2": np.ascontiguousarray(np.asarray(g2, np.float32)),
        "beta2": np.ascontiguousarray(np.asarray(beta2, np.float32)),
    }
    in_maps = [{"xT": np.ascontiguousarray(x[b].T), **common} for b in range(B)]
    res = run_bass_kernel_spmd(nc, in_maps, core_ids=list(range(B)), trace=_trace)
    out = np.stack([res.results[b]["out"].T for b in range(B)])
    if _trace:
        kernel.last_exec_time_ns = res.exec_time_ns
        kernel.last_mean_exec_time_ns = res.mean_exec_time_ns
        kernel.last_results = res
    return out.astype(np.float32)
